# revision 14
# baseline (speedup 1.0000x reference)
"""Trainium2 Bass kernel for nn_Cross_head (sparse_attention patch-correction).

Math (non-overlapping unfold/fold are inverse permutations):
    y   = W @ x + b                   (1x1x1 conv over channels)
    out = leaky_relu(y * (y + 1 + A), 0.2),  A = att / (count_nonzero(att) + 1e-5)

Factorization used on device (q read once from PSUM by the scalar engine):
    q   = psum + (b+1)        # scalar engine, PSUM -> SBUF fp16
    A   = att * r             # r = 1/nz broadcast per patch column
    t   = A + q               # = y + 1 + A
    p   = q - 1               # = y
    pre = t * p
    out = prelu(pre, 0.2)

Sharding: spatial across the 576 patch columns (72 per core), no cross-core
communication.  All I/O is fp16 (host casts/packs), every DMA is contiguous
per channel (5832B descriptors), pure HWDGE on the sync queue.

Per-subtile free layout is (pq=81, p2-major inside, iw=36) so that every
element-wise operand is innermost-packed fp16 in SBUF: TT ops run in DVE 2x
mode, TS ops in 4x mode.  The 1/nz broadcast operand is packed on its
innermost (iw) dim with stride-0 only on the outer pq dim, which keeps 2x.
The nz count-reduce reads st=(att!=0) with a strided view (no fast mode for
reduce anyway).
"""

import os
import sys

import numpy as np

sys.path.insert(0, "/opt/trn_rl_repo")

# ---- geometry (hardcoded for this problem) ----
C = 128          # channels (in == out)
D = 36           # depth
HWFULL = 5184    # H*W = 72*72
PS = 9           # patch size
PQ = PS * PS     # 81 kernel positions
NDP = 4          # D // PS
NWP = 576        # HWFULL // PS  (patch columns)
NCORES = 8
IWG = NWP // NCORES   # 72 patch columns per core
NSUB = 2              # split each iD block into halves along iW
IWT = IWG // NSUB     # 36 patch columns per subtile
FT = IWT * PQ         # 2916 elements per subtile per partition
NT = NDP * NSUB       # 8 subtiles
MMN = 486             # matmul free dim (2916 / 6)
NMM = 6               # matmuls per subtile
NGRP = 2              # psum groups per subtile
MMG = NMM // NGRP     # 3 matmuls per psum group
BANK = 512            # fp32 elements per PSUM bank

_NC_CACHE = {}
LAST_RESULT = None


def _build_nc(ne_engine="vector", amul_engine="gpsimd", prelu_engine="scalar",
              p_engine="vector", nz_engine="fold"):
    from contextlib import ExitStack

    import concourse.bacc as bacc
    import concourse.tile as tile
    from concourse import mybir

    f32 = mybir.dt.float32
    f16 = mybir.dt.float16
    AL = mybir.AluOpType
    AF = mybir.ActivationFunctionType

    nc = bacc.Bacc(
        "TRN2",
        target_bir_lowering=False,
        debug=False,
        enable_asserts=False,
        num_devices=NCORES,
    )
    x_d = nc.dram_tensor("x", [C, NT, FT], f16, kind="ExternalInput").ap()
    a_d = nc.dram_tensor("att", [C, NT, FT], f16, kind="ExternalInput").ap()
    wt_d = nc.dram_tensor("wt", [C, C], f16, kind="ExternalInput").ap()
    id_d = nc.dram_tensor("ident", [C, C], f16, kind="ExternalInput").ap()
    b_d = nc.dram_tensor("bias", [C, 2], f32, kind="ExternalInput").ap()
    o_d = nc.dram_tensor("out", [C, NT, FT], f16, kind="ExternalOutput").ap()

    with tile.TileContext(nc) as tc, ExitStack() as ctx:
        const = ctx.enter_context(tc.tile_pool(name="const", bufs=1))
        wt_sb = const.tile([C, C], f16)
        nc.sync.dma_start(wt_sb[:], wt_d[:])
        id_sb = const.tile([C, C], f16)
        nc.sync.dma_start(id_sb[:], id_d[:])
        b_sb = const.tile([C, 2], f32)
        nc.sync.dma_start(b_sb[:], b_d[:])
        b_ap = b_sb[:, 0:1]
        bp1_ap = b_sb[:, 1:2]
        alpha_sb = const.tile([C, 1], f32)
        nc.vector.memset(alpha_sb[:], 0.2)

        xp = ctx.enter_context(tc.tile_pool(name="xp", bufs=3))
        atp = ctx.enter_context(tc.tile_pool(name="atp", bufs=3))
        stp = ctx.enter_context(tc.tile_pool(name="stp", bufs=2))
        nzp = ctx.enter_context(tc.tile_pool(name="nzp", bufs=2))
        Apl = ctx.enter_context(tc.tile_pool(name="Apl", bufs=2))
        qpl = ctx.enter_context(tc.tile_pool(name="qpl", bufs=2))
        tpl = ctx.enter_context(tc.tile_pool(name="tpl", bufs=2))
        ppl = ctx.enter_context(tc.tile_pool(name="ppl", bufs=2))
        prp = ctx.enter_context(tc.tile_pool(name="prp", bufs=2))
        ovp = ctx.enter_context(tc.tile_pool(name="ovp", bufs=3))
        psp = ctx.enter_context(tc.tile_pool(name="psp", bufs=2, space="PSUM"))
        nzps = (
            ctx.enter_context(tc.tile_pool(name="nzps", bufs=2, space="PSUM"))
            if nz_engine == "tensor"
            else None
        )

        ne_eng = {"vector": nc.vector, "gpsimd": nc.gpsimd}.get(ne_engine)
        amul = {"vector": nc.vector, "gpsimd": nc.gpsimd}[amul_engine]

        def issue_loads(sub):
            xt = xp.tile([C, FT], f16, name=f"xt{sub}", tag="xt")
            nc.sync.dma_start(xt[:], x_d[:, sub, :])
            at = atp.tile([C, FT], f16, name=f"at{sub}", tag="at")
            nc.sync.dma_start(at[:], a_d[:, sub, :])
            return xt, at

        loaded = {s: issue_loads(s) for s in range(3)}

        for sub in range(NT):
            xt, at = loaded.pop(sub)
            if sub + 3 < NT:
                loaded[sub + 3] = issue_loads(sub + 3)

            # ---- nz = count_nonzero per patch column ----
            nzv = nzp.tile([C, IWT], f32, name=f"nz{sub}", tag="nz")
            if nz_engine == "fold":
                # st = (att != 0) at 4x, then a pairwise fold tree of 2x TT
                # adds over the 81 kernel positions (81 = 2*40 + 1), a tiny
                # strided reduce over the last 5 planes, and the leftover
                # plane folded in at [C, 36] size.
                st = stp.tile([C, FT], f16, name=f"st{sub}", tag="st")
                nc.vector.tensor_scalar(st[:], at[:], 0.0, None, AL.not_equal)
                u1 = stp.tile([C, 40 * IWT], f16, name=f"u1{sub}", tag="u1")
                nc.vector.tensor_tensor(
                    u1[:], st[:, : 40 * IWT], st[:, 40 * IWT : 80 * IWT], AL.add
                )
                u2 = u1[:, : 20 * IWT]
                nc.vector.tensor_tensor(
                    u2, u1[:, : 20 * IWT], u1[:, 20 * IWT : 40 * IWT], AL.add
                )
                u3 = u1[:, : 10 * IWT]
                nc.vector.tensor_tensor(
                    u3, u1[:, : 10 * IWT], u1[:, 10 * IWT : 20 * IWT], AL.add
                )
                u4 = u1[:, : 5 * IWT]
                nc.vector.tensor_tensor(
                    u4, u1[:, : 5 * IWT], u1[:, 5 * IWT : 10 * IWT], AL.add
                )
                nz5 = nzp.tile([C, IWT], f32, name=f"n5{sub}", tag="n5")
                nc.vector.tensor_reduce(
                    nz5[:],
                    u1[:, : 5 * IWT]
                    .rearrange("c (q w) -> c q w", q=5)
                    .transpose([0, 2, 1]),
                    mybir.AxisListType.X,
                    AL.add,
                )
                nc.vector.tensor_tensor(
                    nzv[:], nz5[:], st[:, 80 * IWT : 81 * IWT], AL.add
                )
            elif nz_engine == "tensor":
                # st = (att != 0) on DVE (4x); sum over p1 via 9 accumulated
                # identity matmuls on the tensor engine; sum over p2 with a
                # small 324-element DVE reduce out of PSUM.
                st = stp.tile([C, FT], f16, name=f"st{sub}", tag="st")
                nc.vector.tensor_scalar(st[:], at[:], 0.0, None, AL.not_equal)
                nzq = nzps.tile([C, PS * IWT], f32)  # 1 bank
                for p1 in range(PS):
                    nc.tensor.matmul(
                        nzq[:],
                        id_sb[:],
                        st[:, p1 * PS * IWT : (p1 + 1) * PS * IWT],
                        start=(p1 == 0),
                        stop=(p1 == PS - 1),
                    )
                nc.vector.tensor_reduce(
                    nzv[:],
                    nzq[:].rearrange("c (q w) -> c q w", q=PS).transpose([0, 2, 1]),
                    mybir.AxisListType.X,
                    AL.add,
                )
            elif ne_engine == "scalar":
                # |sign(att)| summed with absolute-value reduce
                st = stp.tile([C, FT], f16, name=f"st{sub}", tag="st")
                nc.scalar.activation(st[:], at[:], AF.Sign)
                nc.vector.tensor_reduce(
                    nzv[:],
                    st[:].rearrange("c (q w) -> c q w", q=PQ).transpose([0, 2, 1]),
                    mybir.AxisListType.X,
                    AL.add,
                    apply_absolute_value=True,
                )
            else:
                st = stp.tile([C, FT], f16, name=f"st{sub}", tag="st")
                ne_eng.tensor_scalar(st[:], at[:], 0.0, None, AL.not_equal)
                nc.vector.tensor_reduce(
                    nzv[:],
                    st[:].rearrange("c (q w) -> c q w", q=PQ).transpose([0, 2, 1]),
                    mybir.AxisListType.X,
                    AL.add,
                )
            # r = 1/nz in fp16 (the +1e-5 of the reference shifts r by
            # ~1.2e-7 relative — far below fp16 rounding, so it is dropped)
            rcp = nzp.tile([C, IWT], f32, name=f"rc{sub}", tag="rc")
            nc.vector.reciprocal_approx_fast(rcp[:], nzv[:])
            rh = nzp.tile([C, IWT], f16, name=f"rh{sub}", tag="rh")
            nc.vector.tensor_scalar(rh[:], rcp[:], 0.0, None, AL.add)

            # ---- A = att * r  (r broadcast over the 81 kernel positions) ----
            At = Apl.tile([C, FT], f16, name=f"A{sub}", tag="A")
            a3 = at[:].rearrange("c (q w) -> c q w", q=PQ)
            r3 = rh[:].unsqueeze(1).broadcast_to((C, PQ, IWT))
            amul.tensor_tensor(
                At[:].rearrange("c (q w) -> c q w", q=PQ), a3, r3, AL.mult
            )

            # ---- GEMM: psum = W @ x ----
            pst = []
            for g in range(NGRP):
                ps_t = psp.tile([C, MMG * BANK], f32)  # 3 banks
                pst.append(ps_t)
                for m in range(MMG):
                    ch = g * MMG + m
                    nc.tensor.matmul(
                        ps_t[:, m * BANK : m * BANK + MMN],
                        wt_sb[:],
                        xt[:, ch * MMN : (ch + 1) * MMN],
                        start=True,
                        stop=True,
                    )

            # ---- q = psum + (b+1), PSUM -> fp16 SBUF on the scalar engine --
            qt = qpl.tile([C, FT], f16, name=f"q{sub}", tag="q")
            for g in range(NGRP):
                ps_ap = (
                    pst[g][:]
                    .rearrange("c (m n) -> c m n", n=BANK)[:, :, 0:MMN]
                )
                q_ap = qt[:, g * MMG * MMN : (g + 1) * MMG * MMN].rearrange(
                    "c (m n) -> c m n", n=MMN
                )
                nc.scalar.activation(q_ap, ps_ap, AF.Identity, bias=bp1_ap)

            # ---- t = A + q ; p = q - 1 ; pre = t * p ----
            tt = tpl.tile([C, FT], f16, name=f"t{sub}", tag="t")
            nc.vector.tensor_tensor(tt[:], At[:], qt[:], AL.add)
            pt = ppl.tile([C, FT], f16, name=f"p{sub}", tag="p")
            if p_engine == "vector":
                nc.vector.tensor_scalar(pt[:], qt[:], 1.0, None, AL.subtract)
            else:
                nc.scalar.activation(pt[:], qt[:], AF.Identity, bias=-1.0)
            pre = prp.tile([C, FT], f16, name=f"pr{sub}", tag="pr")
            nc.vector.tensor_tensor(pre[:], tt[:], pt[:], AL.mult)

            # ---- out = lrelu(pre) ----
            ov = ovp.tile([C, FT], f16, name=f"ov{sub}", tag="ov")
            if prelu_engine == "scalar":
                nc.scalar.activation(ov[:], pre[:], AF.Prelu, alpha=alpha_sb[:, 0:1])
            else:
                nc.vector.scalar_tensor_tensor(
                    ov[:], pre[:], 0.2, pre[:], AL.mult, AL.max
                )

            nc.sync.dma_start(o_d[:, sub, :], ov[:])

    nc.compile()
    return nc


def _get_nc(**kw):
    key = tuple(sorted(kw.items()))
    if key not in _NC_CACHE:
        _NC_CACHE[key] = _build_nc(**kw)
    return _NC_CACHE[key]


def kernel(x, attentions, W, b, **build_kw):
    global LAST_RESULT
    from concourse.bass_utils import run_bass_kernel_spmd

    x = np.asarray(x, dtype=np.float32)
    attentions = np.asarray(attentions, dtype=np.float32)
    W = np.asarray(W, dtype=np.float32)
    b = np.asarray(b, dtype=np.float32)

    nc = _get_nc(**build_kw)

    # x: [1, C, D, HW] -> (c, iD, p1, s, h, iw, p2) -> per-core (c, iD, h, p1, p2, iw)
    xs = x.reshape(C, NDP, PS, NCORES, NSUB, IWT, PS)
    # att: [1, C, L, 81] with L=(iD, s, h, iw), 81=(p1, p2)
    as_ = attentions.reshape(C, NDP, NCORES, NSUB, IWT, PS, PS)
    wt = np.ascontiguousarray(W.T.astype(np.float16))
    ident = np.eye(C, dtype=np.float16)
    bcol = np.ascontiguousarray(np.stack([b, b + 1.0], axis=1))

    in_maps = []
    for s in range(NCORES):
        xc = xs[:, :, :, s].transpose(0, 1, 3, 2, 5, 4)  # c,iD,h,p1,p2,iw
        ac = as_[:, :, s].transpose(0, 1, 2, 4, 5, 3)    # c,iD,h,p1,p2,iw
        in_maps.append(
            {
                "x": np.ascontiguousarray(xc, dtype=np.float16).reshape(C, NT, FT),
                "att": np.ascontiguousarray(ac, dtype=np.float16).reshape(C, NT, FT),
                "wt": wt,
                "ident": ident,
                "bias": bcol,
            }
        )

    res = run_bass_kernel_spmd(
        nc,
        in_maps,
        core_ids=list(range(NCORES)),
        trace=bool(os.environ.get("BASS_TRACE")),
    )
    LAST_RESULT = res

    # out: per-core [C, NT, FT] = (c, iD, h, p1, p2, iw) -> [1, C, D, HW]
    full = np.empty((C, NDP, PS, NCORES, NSUB, IWT, PS), dtype=np.float32)
    for s in range(NCORES):
        oc = res.results[s]["out"].reshape(C, NDP, NSUB, PS, PS, IWT)
        full[:, :, :, s] = oc.transpose(0, 1, 3, 2, 5, 4).astype(np.float32)
    return full.reshape(1, C, D, HWFULL)


# revision 17
# speedup vs baseline: 1.0077x; 1.0077x over previous
"""Trainium2 Bass kernel for nn_Cross_head (sparse_attention patch-correction).

Math (non-overlapping unfold/fold are inverse permutations):
    y   = W @ x + b                   (1x1x1 conv over channels)
    out = leaky_relu(y * (y + 1 + A), 0.2),  A = att / (count_nonzero(att) + 1e-5)

Factorization used on device (q read once from PSUM by the scalar engine):
    q   = psum + (b+1)        # scalar engine, PSUM -> SBUF fp16
    A   = att * r             # r = 1/nz broadcast per patch column
    t   = A + q               # = y + 1 + A
    p   = q - 1               # = y
    pre = t * p
    out = prelu(pre, 0.2)

Sharding: spatial across the 576 patch columns (72 per core), no cross-core
communication.  All I/O is fp16 (host casts/packs), every DMA is contiguous
per channel (5832B descriptors), pure HWDGE on the sync queue.

Per-subtile free layout is (pq=81, p2-major inside, iw=36) so that every
element-wise operand is innermost-packed fp16 in SBUF: TT ops run in DVE 2x
mode, TS ops in 4x mode.  The 1/nz broadcast operand is packed on its
innermost (iw) dim with stride-0 only on the outer pq dim, which keeps 2x.
The nz count-reduce reads st=(att!=0) with a strided view (no fast mode for
reduce anyway).
"""

import os
import sys

import numpy as np

sys.path.insert(0, "/opt/trn_rl_repo")

# ---- geometry (hardcoded for this problem) ----
C = 128          # channels (in == out)
D = 36           # depth
HWFULL = 5184    # H*W = 72*72
PS = 9           # patch size
PQ = PS * PS     # 81 kernel positions
NDP = 4          # D // PS
NWP = 576        # HWFULL // PS  (patch columns)
NCORES = 8
IWG = NWP // NCORES   # 72 patch columns per core
NSUB = 2              # split each iD block into halves along iW
IWT = IWG // NSUB     # 36 patch columns per subtile
FT = IWT * PQ         # 2916 elements per subtile per partition
NT = NDP * NSUB       # 8 subtiles
MMN = 486             # matmul free dim (2916 / 6)
NMM = 6               # matmuls per subtile
NGRP = 2              # psum groups per subtile
MMG = NMM // NGRP     # 3 matmuls per psum group
BANK = 512            # fp32 elements per PSUM bank

_NC_CACHE = {}
LAST_RESULT = None


def _build_nc(ne_engine="vector", amul_engine="gpsimd", prelu_engine="scalar",
              p_engine="vector", nz_engine="tensor", ne_mode="ttz"):
    from contextlib import ExitStack

    import concourse.bacc as bacc
    import concourse.tile as tile
    from concourse import mybir

    f32 = mybir.dt.float32
    f16 = mybir.dt.float16
    AL = mybir.AluOpType
    AF = mybir.ActivationFunctionType

    nc = bacc.Bacc(
        "TRN2",
        target_bir_lowering=False,
        debug=False,
        enable_asserts=False,
        num_devices=NCORES,
    )
    x_d = nc.dram_tensor("x", [C, NT, FT], f16, kind="ExternalInput").ap()
    a_d = nc.dram_tensor("att", [C, NT, FT], f16, kind="ExternalInput").ap()
    wt_d = nc.dram_tensor("wt", [C, C], f16, kind="ExternalInput").ap()
    id_d = nc.dram_tensor("ident", [C, C], f16, kind="ExternalInput").ap()
    b_d = nc.dram_tensor("bias", [C, 2], f32, kind="ExternalInput").ap()
    o_d = nc.dram_tensor("out", [C, NT, FT], f16, kind="ExternalOutput").ap()

    with tile.TileContext(nc) as tc, ExitStack() as ctx:
        const = ctx.enter_context(tc.tile_pool(name="const", bufs=1))
        wt_sb = const.tile([C, C], f16)
        nc.sync.dma_start(wt_sb[:], wt_d[:])
        id_sb = const.tile([C, C], f16)
        nc.sync.dma_start(id_sb[:], id_d[:])
        b_sb = const.tile([C, 2], f32)
        nc.sync.dma_start(b_sb[:], b_d[:])
        b_ap = b_sb[:, 0:1]
        bp1_ap = b_sb[:, 1:2]
        alpha_sb = const.tile([C, 1], f32)
        nc.vector.memset(alpha_sb[:], 0.2)
        zt = const.tile([C, FT], f16)
        nc.vector.memset(zt[:], 0.0)

        xp = ctx.enter_context(tc.tile_pool(name="xp", bufs=3))
        atp = ctx.enter_context(tc.tile_pool(name="atp", bufs=3))
        stp = ctx.enter_context(tc.tile_pool(name="stp", bufs=2))
        nzp = ctx.enter_context(tc.tile_pool(name="nzp", bufs=2))
        Apl = ctx.enter_context(tc.tile_pool(name="Apl", bufs=2))
        qpl = ctx.enter_context(tc.tile_pool(name="qpl", bufs=2))
        tpl = ctx.enter_context(tc.tile_pool(name="tpl", bufs=2))
        ppl = ctx.enter_context(tc.tile_pool(name="ppl", bufs=2))
        prp = ctx.enter_context(tc.tile_pool(name="prp", bufs=2))
        ovp = ctx.enter_context(tc.tile_pool(name="ovp", bufs=3))
        psp = ctx.enter_context(tc.tile_pool(name="psp", bufs=2, space="PSUM"))
        nzps = (
            ctx.enter_context(tc.tile_pool(name="nzps", bufs=2, space="PSUM"))
            if nz_engine == "tensor"
            else None
        )

        ne_eng = {"vector": nc.vector, "gpsimd": nc.gpsimd}.get(ne_engine)
        amul = {"vector": nc.vector, "gpsimd": nc.gpsimd}[amul_engine]

        def issue_loads(sub):
            xt = xp.tile([C, FT], f16, name=f"xt{sub}", tag="xt")
            nc.sync.dma_start(xt[:], x_d[:, sub, :])
            at = atp.tile([C, FT], f16, name=f"at{sub}", tag="at")
            nc.sync.dma_start(at[:], a_d[:, sub, :])
            return xt, at

        loaded = {s: issue_loads(s) for s in range(3)}

        for sub in range(NT):
            xt, at = loaded.pop(sub)
            if sub + 3 < NT:
                loaded[sub + 3] = issue_loads(sub + 3)

            # ---- nz = count_nonzero per patch column ----
            nzv = nzp.tile([C, IWT], f32, name=f"nz{sub}", tag="nz")
            if nz_engine == "fold":
                # st = (att != 0) at 4x, then a pairwise fold tree of 2x TT
                # adds over the 81 kernel positions (81 = 2*40 + 1), a tiny
                # strided reduce over the last 5 planes, and the leftover
                # plane folded in at [C, 36] size.
                st = stp.tile([C, FT], f16, name=f"st{sub}", tag="st")
                nc.vector.tensor_scalar(st[:], at[:], 0.0, None, AL.not_equal)
                u1 = stp.tile([C, 40 * IWT], f16, name=f"u1{sub}", tag="u1")
                nc.vector.tensor_tensor(
                    u1[:], st[:, : 40 * IWT], st[:, 40 * IWT : 80 * IWT], AL.add
                )
                u2 = u1[:, : 20 * IWT]
                nc.vector.tensor_tensor(
                    u2, u1[:, : 20 * IWT], u1[:, 20 * IWT : 40 * IWT], AL.add
                )
                u3 = u1[:, : 10 * IWT]
                nc.vector.tensor_tensor(
                    u3, u1[:, : 10 * IWT], u1[:, 10 * IWT : 20 * IWT], AL.add
                )
                u4 = u1[:, : 5 * IWT]
                nc.vector.tensor_tensor(
                    u4, u1[:, : 5 * IWT], u1[:, 5 * IWT : 10 * IWT], AL.add
                )
                nz5 = nzp.tile([C, IWT], f32, name=f"n5{sub}", tag="n5")
                nc.vector.tensor_reduce(
                    nz5[:],
                    u1[:, : 5 * IWT]
                    .rearrange("c (q w) -> c q w", q=5)
                    .transpose([0, 2, 1]),
                    mybir.AxisListType.X,
                    AL.add,
                )
                nc.vector.tensor_tensor(
                    nzv[:], nz5[:], st[:, 80 * IWT : 81 * IWT], AL.add
                )
            elif nz_engine == "tensor":
                # st = (att != 0) on DVE; sum over p1 via 9 accumulated
                # identity matmuls on the tensor engine; sum over p2 with a
                # small 324-element DVE reduce out of PSUM.  The != test runs
                # as TT against a zeros tile (2x, port-light) by default: 4x
                # TS ops starve on SBUF ports when the PE streams heavily.
                st = stp.tile([C, FT], f16, name=f"st{sub}", tag="st")
                if ne_mode == "ttz":
                    nc.vector.tensor_tensor(st[:], at[:], zt[:], AL.not_equal)
                else:
                    nc.vector.tensor_scalar(st[:], at[:], 0.0, None, AL.not_equal)
                nzq = nzps.tile([C, PS * IWT], f32)  # 1 bank
                for p1 in range(PS):
                    nc.tensor.matmul(
                        nzq[:],
                        id_sb[:],
                        st[:, p1 * PS * IWT : (p1 + 1) * PS * IWT],
                        start=(p1 == 0),
                        stop=(p1 == PS - 1),
                    )
                nc.vector.tensor_reduce(
                    nzv[:],
                    nzq[:].rearrange("c (q w) -> c q w", q=PS).transpose([0, 2, 1]),
                    mybir.AxisListType.X,
                    AL.add,
                )
            elif ne_engine == "scalar":
                # |sign(att)| summed with absolute-value reduce
                st = stp.tile([C, FT], f16, name=f"st{sub}", tag="st")
                nc.scalar.activation(st[:], at[:], AF.Sign)
                nc.vector.tensor_reduce(
                    nzv[:],
                    st[:].rearrange("c (q w) -> c q w", q=PQ).transpose([0, 2, 1]),
                    mybir.AxisListType.X,
                    AL.add,
                    apply_absolute_value=True,
                )
            else:
                st = stp.tile([C, FT], f16, name=f"st{sub}", tag="st")
                ne_eng.tensor_scalar(st[:], at[:], 0.0, None, AL.not_equal)
                nc.vector.tensor_reduce(
                    nzv[:],
                    st[:].rearrange("c (q w) -> c q w", q=PQ).transpose([0, 2, 1]),
                    mybir.AxisListType.X,
                    AL.add,
                )
            # r = 1/nz in fp16 (the +1e-5 of the reference shifts r by
            # ~1.2e-7 relative — far below fp16 rounding, so it is dropped)
            rcp = nzp.tile([C, IWT], f32, name=f"rc{sub}", tag="rc")
            nc.vector.reciprocal_approx_fast(rcp[:], nzv[:])
            rh = nzp.tile([C, IWT], f16, name=f"rh{sub}", tag="rh")
            nc.vector.tensor_scalar(rh[:], rcp[:], 0.0, None, AL.add)

            # ---- A = att * r  (r broadcast over the 81 kernel positions) ----
            At = Apl.tile([C, FT], f16, name=f"A{sub}", tag="A")
            a3 = at[:].rearrange("c (q w) -> c q w", q=PQ)
            r3 = rh[:].unsqueeze(1).broadcast_to((C, PQ, IWT))
            amul.tensor_tensor(
                At[:].rearrange("c (q w) -> c q w", q=PQ), a3, r3, AL.mult
            )

            # ---- GEMM: psum = W @ x ----
            pst = []
            for g in range(NGRP):
                ps_t = psp.tile([C, MMG * BANK], f32)  # 3 banks
                pst.append(ps_t)
                for m in range(MMG):
                    ch = g * MMG + m
                    nc.tensor.matmul(
                        ps_t[:, m * BANK : m * BANK + MMN],
                        wt_sb[:],
                        xt[:, ch * MMN : (ch + 1) * MMN],
                        start=True,
                        stop=True,
                    )

            # ---- q = psum + (b+1), PSUM -> fp16 SBUF on the scalar engine --
            qt = qpl.tile([C, FT], f16, name=f"q{sub}", tag="q")
            for g in range(NGRP):
                ps_ap = (
                    pst[g][:]
                    .rearrange("c (m n) -> c m n", n=BANK)[:, :, 0:MMN]
                )
                q_ap = qt[:, g * MMG * MMN : (g + 1) * MMG * MMN].rearrange(
                    "c (m n) -> c m n", n=MMN
                )
                nc.scalar.activation(q_ap, ps_ap, AF.Identity, bias=bp1_ap)

            # ---- t = A + q ; p = q - 1 ; pre = t * p ----
            tt = tpl.tile([C, FT], f16, name=f"t{sub}", tag="t")
            nc.vector.tensor_tensor(tt[:], At[:], qt[:], AL.add)
            pt = ppl.tile([C, FT], f16, name=f"p{sub}", tag="p")
            if p_engine == "vector":
                nc.vector.tensor_scalar(pt[:], qt[:], 1.0, None, AL.subtract)
            else:
                nc.scalar.activation(pt[:], qt[:], AF.Identity, bias=-1.0)
            pre = prp.tile([C, FT], f16, name=f"pr{sub}", tag="pr")
            nc.vector.tensor_tensor(pre[:], tt[:], pt[:], AL.mult)

            # ---- out = lrelu(pre) ----
            ov = ovp.tile([C, FT], f16, name=f"ov{sub}", tag="ov")
            if prelu_engine == "scalar":
                nc.scalar.activation(ov[:], pre[:], AF.Prelu, alpha=alpha_sb[:, 0:1])
            else:
                nc.vector.scalar_tensor_tensor(
                    ov[:], pre[:], 0.2, pre[:], AL.mult, AL.max
                )

            nc.sync.dma_start(o_d[:, sub, :], ov[:])

    nc.compile()
    return nc


def _get_nc(**kw):
    key = tuple(sorted(kw.items()))
    if key not in _NC_CACHE:
        _NC_CACHE[key] = _build_nc(**kw)
    return _NC_CACHE[key]


def kernel(x, attentions, W, b, **build_kw):
    global LAST_RESULT
    from concourse.bass_utils import run_bass_kernel_spmd

    x = np.asarray(x, dtype=np.float32)
    attentions = np.asarray(attentions, dtype=np.float32)
    W = np.asarray(W, dtype=np.float32)
    b = np.asarray(b, dtype=np.float32)

    nc = _get_nc(**build_kw)

    # x: [1, C, D, HW] -> (c, iD, p1, s, h, iw, p2) -> per-core (c, iD, h, p1, p2, iw)
    xs = x.reshape(C, NDP, PS, NCORES, NSUB, IWT, PS)
    # att: [1, C, L, 81] with L=(iD, s, h, iw), 81=(p1, p2)
    as_ = attentions.reshape(C, NDP, NCORES, NSUB, IWT, PS, PS)
    wt = np.ascontiguousarray(W.T.astype(np.float16))
    ident = np.eye(C, dtype=np.float16)
    bcol = np.ascontiguousarray(np.stack([b, b + 1.0], axis=1))

    in_maps = []
    for s in range(NCORES):
        xc = xs[:, :, :, s].transpose(0, 1, 3, 2, 5, 4)  # c,iD,h,p1,p2,iw
        ac = as_[:, :, s].transpose(0, 1, 2, 4, 5, 3)    # c,iD,h,p1,p2,iw
        in_maps.append(
            {
                "x": np.ascontiguousarray(xc, dtype=np.float16).reshape(C, NT, FT),
                "att": np.ascontiguousarray(ac, dtype=np.float16).reshape(C, NT, FT),
                "wt": wt,
                "ident": ident,
                "bias": bcol,
            }
        )

    res = run_bass_kernel_spmd(
        nc,
        in_maps,
        core_ids=list(range(NCORES)),
        trace=bool(os.environ.get("BASS_TRACE")),
    )
    LAST_RESULT = res

    # out: per-core [C, NT, FT] = (c, iD, h, p1, p2, iw) -> [1, C, D, HW]
    full = np.empty((C, NDP, PS, NCORES, NSUB, IWT, PS), dtype=np.float32)
    for s in range(NCORES):
        oc = res.results[s]["out"].reshape(C, NDP, NSUB, PS, PS, IWT)
        full[:, :, :, s] = oc.transpose(0, 1, 3, 2, 5, 4).astype(np.float32)
    return full.reshape(1, C, D, HWFULL)


# revision 22
# speedup vs baseline: 1.2212x; 1.2119x over previous
"""Trainium2 Bass kernel for nn_Cross_head (sparse_attention patch-correction).

Math (non-overlapping unfold/fold are inverse permutations):
    y   = W @ x + b                   (1x1x1 conv over channels)
    out = leaky_relu(y * (y + 1 + A), 0.2),  A = att / (count_nonzero(att) + 1e-5)

Factorization used on device (q read once from PSUM by the scalar engine):
    q   = psum + (b+1)        # scalar engine, PSUM -> SBUF fp16
    A   = att * r             # r = 1/nz broadcast per patch column
    t   = A + q               # = y + 1 + A
    p   = q - 1               # = y
    pre = t * p
    out = prelu(pre, 0.2)

Sharding: spatial across the 576 patch columns (72 per core), no cross-core
communication.  All I/O is fp16 (host casts/packs), every DMA is contiguous
per channel (5832B descriptors), pure HWDGE on the sync queue.

Per-subtile free layout is (pq=81, p2-major inside, iw=36) so that every
element-wise operand is innermost-packed fp16 in SBUF: TT ops run in DVE 2x
mode, TS ops in 4x mode.  The 1/nz broadcast operand is packed on its
innermost (iw) dim with stride-0 only on the outer pq dim, which keeps 2x.
The nz count-reduce reads st=(att!=0) with a strided view (no fast mode for
reduce anyway).
"""

import os
import sys

import numpy as np

sys.path.insert(0, "/opt/trn_rl_repo")

# ---- geometry (hardcoded for this problem) ----
C = 128          # channels (in == out)
D = 36           # depth
HWFULL = 5184    # H*W = 72*72
PS = 9           # patch size
PQ = PS * PS     # 81 kernel positions
NDP = 4          # D // PS
NWP = 576        # HWFULL // PS  (patch columns)
NCORES = 8
IWG = NWP // NCORES   # 72 patch columns per core
NSUB = 2              # split each iD block into halves along iW
IWT = IWG // NSUB     # 36 patch columns per subtile
FT = IWT * PQ         # 2916 elements per subtile per partition
NT = NDP * NSUB       # 8 subtiles
MMN = 486             # matmul free dim (2916 / 6)
NMM = 6               # matmuls per subtile
NGRP = 2              # psum groups per subtile
MMG = NMM // NGRP     # 3 matmuls per psum group
BANK = 512            # fp32 elements per PSUM bank

_NC_CACHE = {}
LAST_RESULT = None

_OP_PRE = None


def _get_op_pre():
    """Register (once) a custom DVE op that fuses the whole element-wise
    tail into a single pass reading PSUM:
        z0  = in0 + s0          # psum + (b+1)  -> y + 1 + A-part. . .
        z   = (z0 + in1) * (z0 - s1)   # (y+1+A) * y   [s1 = 1]
        out = max(z, z*imm2)           # leaky_relu    [imm2 = 0.2]
    This replaces the q/t/p/pre intermediate tiles (and the scalar-engine
    Prelu) with one DVE instruction per PSUM group, saving ~45KB/partition
    of SBUF traffic per subtile.  Uses the documented custom-DVE extension
    point (dve_ops.OPS)."""
    global _OP_PRE
    if _OP_PRE is not None:
        return _OP_PRE
    import numpy as np

    from concourse import dve_ops
    from concourse.dve_spec import C0, C1, C2, Spec, Src0, Src1, lower, maxx
    from concourse.dve_uop import DveOpSpec

    NAME = "CROSS_HEAD_PRE_LRELU"
    for op in dve_ops.OPS:
        if op.name == NAME:
            _OP_PRE = op
            return op

    _z0 = Src0 + C0
    _z = (_z0 + Src1) * (_z0 - C1)
    body = maxx(_z, _z * C2)

    def _ref(in0, in1, s0, s1, imm2):
        p = in0.shape[0]
        a = np.asarray(in0, np.float32).reshape(p, -1)
        b = np.asarray(in1, np.float32).reshape(p, -1)
        z0 = a + s0
        z = (z0 + b) * (z0 - s1)
        return np.maximum(z, z * imm2).astype(np.float32)

    spec = Spec(body=body, reference=_ref)
    row = max(dve_ops._SUB_OPCODE_FOR_NAME.values()) + 1
    assert row < 0x20
    dve_ops._SUB_OPCODE_FOR_NAME[NAME] = row
    shas = {}
    for ver in ("v3", "v4"):
        uops = lower(spec, ver=ver)
        shas[ver] = DveOpSpec(
            name=NAME, opcode=row, uops=uops, rd1_en=True
        ).sha(ver)
    op = dve_ops.DveOp(NAME, spec, subdim=False, uops_sha=shas)
    dve_ops.OPS.append(op)
    dve_ops.CUSTOM_DVE_SPECS[NAME] = spec
    _OP_PRE = op
    return op


def _build_nc(ne_engine="vector", amul_engine="gpsimd", prelu_engine="scalar",
              p_engine="vector", nz_engine="tensor", ne_mode="ttz", fuse="dve"):
    from contextlib import ExitStack

    import concourse.bacc as bacc
    import concourse.tile as tile
    from concourse import mybir

    f32 = mybir.dt.float32
    f16 = mybir.dt.float16
    AL = mybir.AluOpType
    AF = mybir.ActivationFunctionType

    op_pre = _get_op_pre() if fuse == "dve" else None

    nc = bacc.Bacc(
        "TRN2",
        target_bir_lowering=False,
        debug=False,
        enable_asserts=False,
        num_devices=NCORES,
    )
    x_d = nc.dram_tensor("x", [C, NT, FT], f16, kind="ExternalInput").ap()
    a_d = nc.dram_tensor("att", [C, NT, FT], f16, kind="ExternalInput").ap()
    wt_d = nc.dram_tensor("wt", [C, C], f16, kind="ExternalInput").ap()
    id_d = nc.dram_tensor("ident", [C, C], f16, kind="ExternalInput").ap()
    b_d = nc.dram_tensor("bias", [C, 2], f32, kind="ExternalInput").ap()
    o_d = nc.dram_tensor("out", [C, NT, FT], f16, kind="ExternalOutput").ap()

    with tile.TileContext(nc) as tc, ExitStack() as ctx:
        const = ctx.enter_context(tc.tile_pool(name="const", bufs=1))
        wt_sb = const.tile([C, C], f16)
        nc.sync.dma_start(wt_sb[:], wt_d[:])
        id_sb = const.tile([C, C], f16)
        nc.sync.dma_start(id_sb[:], id_d[:])
        b_sb = const.tile([C, 2], f32)
        nc.sync.dma_start(b_sb[:], b_d[:])
        b_ap = b_sb[:, 0:1]
        bp1_ap = b_sb[:, 1:2]
        alpha_sb = const.tile([C, 1], f32)
        nc.vector.memset(alpha_sb[:], 0.2)
        zt = const.tile([C, FT], f16)
        nc.vector.memset(zt[:], 0.0)

        xp = ctx.enter_context(tc.tile_pool(name="xp", bufs=3))
        atp = ctx.enter_context(tc.tile_pool(name="atp", bufs=3))
        stp = ctx.enter_context(tc.tile_pool(name="stp", bufs=2))
        nzp = ctx.enter_context(tc.tile_pool(name="nzp", bufs=2))
        Apl = ctx.enter_context(tc.tile_pool(name="Apl", bufs=2))
        qpl = ctx.enter_context(tc.tile_pool(name="qpl", bufs=2))
        tpl = ctx.enter_context(tc.tile_pool(name="tpl", bufs=2))
        ppl = ctx.enter_context(tc.tile_pool(name="ppl", bufs=2))
        prp = ctx.enter_context(tc.tile_pool(name="prp", bufs=2))
        ovp = ctx.enter_context(tc.tile_pool(name="ovp", bufs=3))
        psp = ctx.enter_context(tc.tile_pool(name="psp", bufs=2, space="PSUM"))
        nzps = (
            ctx.enter_context(tc.tile_pool(name="nzps", bufs=2, space="PSUM"))
            if nz_engine == "tensor"
            else None
        )

        ne_eng = {"vector": nc.vector, "gpsimd": nc.gpsimd}.get(ne_engine)
        amul = {"vector": nc.vector, "gpsimd": nc.gpsimd}[amul_engine]

        def issue_loads(sub):
            xt = xp.tile([C, FT], f16, name=f"xt{sub}", tag="xt")
            nc.sync.dma_start(xt[:], x_d[:, sub, :])
            at = atp.tile([C, FT], f16, name=f"at{sub}", tag="at")
            nc.sync.dma_start(at[:], a_d[:, sub, :])
            return xt, at

        loaded = {s: issue_loads(s) for s in range(3)}

        for sub in range(NT):
            xt, at = loaded.pop(sub)
            if sub + 3 < NT:
                loaded[sub + 3] = issue_loads(sub + 3)

            # ---- nz = count_nonzero per patch column ----
            nzv = nzp.tile([C, IWT], f32, name=f"nz{sub}", tag="nz")
            if nz_engine == "fold":
                # st = (att != 0) at 4x, then a pairwise fold tree of 2x TT
                # adds over the 81 kernel positions (81 = 2*40 + 1), a tiny
                # strided reduce over the last 5 planes, and the leftover
                # plane folded in at [C, 36] size.
                st = stp.tile([C, FT], f16, name=f"st{sub}", tag="st")
                nc.vector.tensor_scalar(st[:], at[:], 0.0, None, AL.not_equal)
                u1 = stp.tile([C, 40 * IWT], f16, name=f"u1{sub}", tag="u1")
                nc.vector.tensor_tensor(
                    u1[:], st[:, : 40 * IWT], st[:, 40 * IWT : 80 * IWT], AL.add
                )
                u2 = u1[:, : 20 * IWT]
                nc.vector.tensor_tensor(
                    u2, u1[:, : 20 * IWT], u1[:, 20 * IWT : 40 * IWT], AL.add
                )
                u3 = u1[:, : 10 * IWT]
                nc.vector.tensor_tensor(
                    u3, u1[:, : 10 * IWT], u1[:, 10 * IWT : 20 * IWT], AL.add
                )
                u4 = u1[:, : 5 * IWT]
                nc.vector.tensor_tensor(
                    u4, u1[:, : 5 * IWT], u1[:, 5 * IWT : 10 * IWT], AL.add
                )
                nz5 = nzp.tile([C, IWT], f32, name=f"n5{sub}", tag="n5")
                nc.vector.tensor_reduce(
                    nz5[:],
                    u1[:, : 5 * IWT]
                    .rearrange("c (q w) -> c q w", q=5)
                    .transpose([0, 2, 1]),
                    mybir.AxisListType.X,
                    AL.add,
                )
                nc.vector.tensor_tensor(
                    nzv[:], nz5[:], st[:, 80 * IWT : 81 * IWT], AL.add
                )
            elif nz_engine == "tensor":
                # st = (att != 0) on DVE; sum over p1 via 9 accumulated
                # identity matmuls on the tensor engine; sum over p2 with a
                # small 324-element DVE reduce out of PSUM.  The != test runs
                # as TT against a zeros tile (2x, port-light) by default: 4x
                # TS ops starve on SBUF ports when the PE streams heavily.
                st = stp.tile([C, FT], f16, name=f"st{sub}", tag="st")
                if ne_mode == "ttz":
                    nc.vector.tensor_tensor(st[:], at[:], zt[:], AL.not_equal)
                else:
                    nc.vector.tensor_scalar(st[:], at[:], 0.0, None, AL.not_equal)
                nzq = nzps.tile([C, PS * IWT], f32)  # 1 bank
                for p1 in range(PS):
                    nc.tensor.matmul(
                        nzq[:],
                        id_sb[:],
                        st[:, p1 * PS * IWT : (p1 + 1) * PS * IWT],
                        start=(p1 == 0),
                        stop=(p1 == PS - 1),
                    )
                nc.vector.tensor_reduce(
                    nzv[:],
                    nzq[:].rearrange("c (q w) -> c q w", q=PS).transpose([0, 2, 1]),
                    mybir.AxisListType.X,
                    AL.add,
                )
            elif nz_engine == "vpad":
                # st padded to pitch 40 (80B rows) so the per-patch reduce
                # walks an aligned stride instead of 72B.
                stw = stp.tile([C, PQ * 40], f16, name=f"sw{sub}", tag="sw")
                st3 = stw[:].rearrange("c (q w) -> c q w", w=40)[:, :, 0:IWT]
                at3 = at[:].rearrange("c (q w) -> c q w", q=PQ)
                if ne_mode == "ttz":
                    zt3 = zt[:].rearrange("c (q w) -> c q w", q=PQ)
                    nc.vector.tensor_tensor(st3, at3, zt3, AL.not_equal)
                else:
                    nc.vector.tensor_scalar(st3, at3, 0.0, None, AL.not_equal)
                nc.vector.tensor_reduce(
                    nzv[:],
                    stw[:].rearrange("c (q w) -> c q w", w=40)[:, :, 0:IWT]
                    .transpose([0, 2, 1]),
                    mybir.AxisListType.X,
                    AL.add,
                )
            elif ne_engine == "scalar":
                # |sign(att)| summed with absolute-value reduce
                st = stp.tile([C, FT], f16, name=f"st{sub}", tag="st")
                nc.scalar.activation(st[:], at[:], AF.Sign)
                nc.vector.tensor_reduce(
                    nzv[:],
                    st[:].rearrange("c (q w) -> c q w", q=PQ).transpose([0, 2, 1]),
                    mybir.AxisListType.X,
                    AL.add,
                    apply_absolute_value=True,
                )
            else:
                st = stp.tile([C, FT], f16, name=f"st{sub}", tag="st")
                ne_eng.tensor_scalar(st[:], at[:], 0.0, None, AL.not_equal)
                nc.vector.tensor_reduce(
                    nzv[:],
                    st[:].rearrange("c (q w) -> c q w", q=PQ).transpose([0, 2, 1]),
                    mybir.AxisListType.X,
                    AL.add,
                )
            # r = 1/nz in fp16 (the +1e-5 of the reference shifts r by
            # ~1.2e-7 relative — far below fp16 rounding, so it is dropped)
            rcp = nzp.tile([C, IWT], f32, name=f"rc{sub}", tag="rc")
            nc.vector.reciprocal_approx_fast(rcp[:], nzv[:])
            rh = nzp.tile([C, IWT], f16, name=f"rh{sub}", tag="rh")
            nc.vector.tensor_scalar(rh[:], rcp[:], 0.0, None, AL.add)

            # ---- A = att * r  (r broadcast over the 81 kernel positions) ----
            At = Apl.tile([C, FT], f16, name=f"A{sub}", tag="A")
            a3 = at[:].rearrange("c (q w) -> c q w", q=PQ)
            r3 = rh[:].unsqueeze(1).broadcast_to((C, PQ, IWT))
            amul.tensor_tensor(
                At[:].rearrange("c (q w) -> c q w", q=PQ), a3, r3, AL.mult
            )

            # ---- GEMM: psum = W @ x ----
            pst = []
            for g in range(NGRP):
                ps_t = psp.tile([C, MMG * BANK], f32)  # 3 banks
                pst.append(ps_t)
                for m in range(MMG):
                    ch = g * MMG + m
                    nc.tensor.matmul(
                        ps_t[:, m * BANK : m * BANK + MMN],
                        wt_sb[:],
                        xt[:, ch * MMN : (ch + 1) * MMN],
                        start=True,
                        stop=True,
                    )

            ov = ovp.tile([C, FT], f16, name=f"ov{sub}", tag="ov")
            if fuse == "dve":
                # ---- ov = lrelu((ps+b+1+A)*(ps+b)) fused, one op/group ----
                for g in range(NGRP):
                    ps_ap = (
                        pst[g][:]
                        .rearrange("c (m n) -> c m n", n=BANK)[:, :, 0:MMN]
                    )
                    sl = slice(g * MMG * MMN, (g + 1) * MMG * MMN)
                    nc.vector._custom_dve(
                        op_pre,
                        out=ov[:, sl],
                        in0=ps_ap,
                        in1=At[:, sl],
                        s0=bp1_ap,
                        s1=1.0,
                        imm2=0.2,
                    )
            else:
                # ---- q = psum + (b+1), PSUM -> fp16 SBUF (scalar engine) ---
                qt = qpl.tile([C, FT], f16, name=f"q{sub}", tag="q")
                for g in range(NGRP):
                    ps_ap = (
                        pst[g][:]
                        .rearrange("c (m n) -> c m n", n=BANK)[:, :, 0:MMN]
                    )
                    q_ap = qt[:, g * MMG * MMN : (g + 1) * MMG * MMN].rearrange(
                        "c (m n) -> c m n", n=MMN
                    )
                    nc.scalar.activation(q_ap, ps_ap, AF.Identity, bias=bp1_ap)

                # ---- t = A + q ; p = q - 1 ; pre = t * p ----
                tt = tpl.tile([C, FT], f16, name=f"t{sub}", tag="t")
                nc.vector.tensor_tensor(tt[:], At[:], qt[:], AL.add)
                pt = ppl.tile([C, FT], f16, name=f"p{sub}", tag="p")
                if p_engine == "vector":
                    nc.vector.tensor_scalar(pt[:], qt[:], 1.0, None, AL.subtract)
                else:
                    nc.scalar.activation(pt[:], qt[:], AF.Identity, bias=-1.0)
                pre = prp.tile([C, FT], f16, name=f"pr{sub}", tag="pr")
                nc.vector.tensor_tensor(pre[:], tt[:], pt[:], AL.mult)

                # ---- out = lrelu(pre) ----
                if prelu_engine == "scalar":
                    nc.scalar.activation(
                        ov[:], pre[:], AF.Prelu, alpha=alpha_sb[:, 0:1]
                    )
                else:
                    nc.vector.scalar_tensor_tensor(
                        ov[:], pre[:], 0.2, pre[:], AL.mult, AL.max
                    )

            nc.sync.dma_start(o_d[:, sub, :], ov[:])

    nc.compile()
    return nc


def _get_nc(**kw):
    key = tuple(sorted(kw.items()))
    if key not in _NC_CACHE:
        _NC_CACHE[key] = _build_nc(**kw)
    return _NC_CACHE[key]


def kernel(x, attentions, W, b, **build_kw):
    global LAST_RESULT
    from concourse.bass_utils import run_bass_kernel_spmd

    x = np.asarray(x, dtype=np.float32)
    attentions = np.asarray(attentions, dtype=np.float32)
    W = np.asarray(W, dtype=np.float32)
    b = np.asarray(b, dtype=np.float32)

    nc = _get_nc(**build_kw)

    # x: [1, C, D, HW] -> (c, iD, p1, s, h, iw, p2) -> per-core (c, iD, h, p1, p2, iw)
    xs = x.reshape(C, NDP, PS, NCORES, NSUB, IWT, PS)
    # att: [1, C, L, 81] with L=(iD, s, h, iw), 81=(p1, p2)
    as_ = attentions.reshape(C, NDP, NCORES, NSUB, IWT, PS, PS)
    wt = np.ascontiguousarray(W.T.astype(np.float16))
    ident = np.eye(C, dtype=np.float16)
    bcol = np.ascontiguousarray(np.stack([b, b + 1.0], axis=1))

    in_maps = []
    for s in range(NCORES):
        xc = xs[:, :, :, s].transpose(0, 1, 3, 2, 5, 4)  # c,iD,h,p1,p2,iw
        ac = as_[:, :, s].transpose(0, 1, 2, 4, 5, 3)    # c,iD,h,p1,p2,iw
        in_maps.append(
            {
                "x": np.ascontiguousarray(xc, dtype=np.float16).reshape(C, NT, FT),
                "att": np.ascontiguousarray(ac, dtype=np.float16).reshape(C, NT, FT),
                "wt": wt,
                "ident": ident,
                "bias": bcol,
            }
        )

    res = run_bass_kernel_spmd(
        nc,
        in_maps,
        core_ids=list(range(NCORES)),
        trace=bool(os.environ.get("BASS_TRACE")),
    )
    LAST_RESULT = res

    # out: per-core [C, NT, FT] = (c, iD, h, p1, p2, iw) -> [1, C, D, HW]
    full = np.empty((C, NDP, PS, NCORES, NSUB, IWT, PS), dtype=np.float32)
    for s in range(NCORES):
        oc = res.results[s]["out"].reshape(C, NDP, NSUB, PS, PS, IWT)
        full[:, :, :, s] = oc.transpose(0, 1, 3, 2, 5, 4).astype(np.float32)
    return full.reshape(1, C, D, HWFULL)


# revision 27
# speedup vs baseline: 1.3296x; 1.0888x over previous
"""Trainium2 Bass kernel for nn_Cross_head (sparse_attention patch-correction).

Math (non-overlapping unfold/fold are inverse permutations):
    y   = W @ x + b                   (1x1x1 conv over channels)
    out = leaky_relu(y * (y + 1 + A), 0.2),  A = att / (count_nonzero(att) + 1e-5)

Factorization used on device (q read once from PSUM by the scalar engine):
    q   = psum + (b+1)        # scalar engine, PSUM -> SBUF fp16
    A   = att * r             # r = 1/nz broadcast per patch column
    t   = A + q               # = y + 1 + A
    p   = q - 1               # = y
    pre = t * p
    out = prelu(pre, 0.2)

Sharding: spatial across the 576 patch columns (72 per core), no cross-core
communication.  All I/O is fp16 (host casts/packs), every DMA is contiguous
per channel (5832B descriptors), pure HWDGE on the sync queue.

Per-subtile free layout is (pq=81, p2-major inside, iw=36) so that every
element-wise operand is innermost-packed fp16 in SBUF: TT ops run in DVE 2x
mode, TS ops in 4x mode.  The 1/nz broadcast operand is packed on its
innermost (iw) dim with stride-0 only on the outer pq dim, which keeps 2x.
The nz count-reduce reads st=(att!=0) with a strided view (no fast mode for
reduce anyway).
"""

import os
import sys

import numpy as np

sys.path.insert(0, "/opt/trn_rl_repo")

# ---- geometry (hardcoded for this problem) ----
C = 128          # channels (in == out)
D = 36           # depth
HWFULL = 5184    # H*W = 72*72
PS = 9           # patch size
PQ = PS * PS     # 81 kernel positions
NDP = 4          # D // PS
NWP = 576        # HWFULL // PS  (patch columns)
NCORES = 8
IWG = NWP // NCORES   # 72 patch columns per core
NSUB = 2              # split each iD block into halves along iW
IWT = IWG // NSUB     # 36 patch columns per subtile
FT = IWT * PQ         # 2916 elements per subtile per partition
NT = NDP * NSUB       # 8 subtiles
MMN = 486             # matmul free dim (2916 / 6)
NMM = 6               # matmuls per subtile
NGRP = 2              # psum groups per subtile
MMG = NMM // NGRP     # 3 matmuls per psum group
BANK = 512            # fp32 elements per PSUM bank

_NC_CACHE = {}
LAST_RESULT = None

_OP_PRE = None


def _get_op_pre():
    """Register (once) a custom DVE op that fuses the whole element-wise
    tail into a single pass reading PSUM:
        z0  = in0 + s0          # psum + (b+1)  -> y + 1 + A-part. . .
        z   = (z0 + in1) * (z0 - s1)   # (y+1+A) * y   [s1 = 1]
        out = max(z, z*imm2)           # leaky_relu    [imm2 = 0.2]
    This replaces the q/t/p/pre intermediate tiles (and the scalar-engine
    Prelu) with one DVE instruction per PSUM group, saving ~45KB/partition
    of SBUF traffic per subtile.  Uses the documented custom-DVE extension
    point (dve_ops.OPS)."""
    global _OP_PRE
    if _OP_PRE is not None:
        return _OP_PRE
    import numpy as np

    from concourse import dve_ops
    from concourse.dve_spec import C0, C1, C2, Spec, Src0, Src1, lower, maxx
    from concourse.dve_uop import DveOpSpec

    NAME = "CROSS_HEAD_PRE_LRELU"
    for op in dve_ops.OPS:
        if op.name == NAME:
            _OP_PRE = op
            return op

    _z0 = Src0 + C0
    _z = (_z0 + Src1) * (_z0 - C1)
    body = maxx(_z, _z * C2)

    def _ref(in0, in1, s0, s1, imm2):
        p = in0.shape[0]
        a = np.asarray(in0, np.float32).reshape(p, -1)
        b = np.asarray(in1, np.float32).reshape(p, -1)
        z0 = a + s0
        z = (z0 + b) * (z0 - s1)
        return np.maximum(z, z * imm2).astype(np.float32)

    spec = Spec(body=body, reference=_ref)
    row = max(dve_ops._SUB_OPCODE_FOR_NAME.values()) + 1
    assert row < 0x20
    dve_ops._SUB_OPCODE_FOR_NAME[NAME] = row
    shas = {}
    for ver in ("v3", "v4"):
        uops = lower(spec, ver=ver)
        shas[ver] = DveOpSpec(
            name=NAME, opcode=row, uops=uops, rd1_en=True
        ).sha(ver)
    op = dve_ops.DveOp(NAME, spec, subdim=False, uops_sha=shas)
    dve_ops.OPS.append(op)
    dve_ops.CUSTOM_DVE_SPECS[NAME] = spec
    _OP_PRE = op
    return op


def _build_nc(ne_engine="vector", amul_engine="gpsimd", prelu_engine="scalar",
              p_engine="vector", nz_engine="tensor", ne_mode="ts", fuse="dve",
              rh_engine="scalar", prefetch=4):
    from contextlib import ExitStack

    import concourse.bacc as bacc
    import concourse.tile as tile
    from concourse import mybir

    f32 = mybir.dt.float32
    f16 = mybir.dt.float16
    AL = mybir.AluOpType
    AF = mybir.ActivationFunctionType

    op_pre = _get_op_pre() if fuse == "dve" else None

    nc = bacc.Bacc(
        "TRN2",
        target_bir_lowering=False,
        debug=False,
        enable_asserts=False,
        num_devices=NCORES,
    )
    x_d = nc.dram_tensor("x", [C, NT, FT], f16, kind="ExternalInput").ap()
    a_d = nc.dram_tensor("att", [C, NT, FT], f16, kind="ExternalInput").ap()
    wt_d = nc.dram_tensor("wt", [C, C], f16, kind="ExternalInput").ap()
    id_d = nc.dram_tensor("ident", [C, C], f16, kind="ExternalInput").ap()
    b_d = nc.dram_tensor("bias", [C, 2], f32, kind="ExternalInput").ap()
    o_d = nc.dram_tensor("out", [C, NT, FT], f16, kind="ExternalOutput").ap()

    with tile.TileContext(nc) as tc, ExitStack() as ctx:
        const = ctx.enter_context(tc.tile_pool(name="const", bufs=1))
        wt_sb = const.tile([C, C], f16)
        nc.sync.dma_start(wt_sb[:], wt_d[:])
        id_sb = const.tile([C, C], f16)
        nc.sync.dma_start(id_sb[:], id_d[:])
        b_sb = const.tile([C, 2], f32)
        nc.sync.dma_start(b_sb[:], b_d[:])
        b_ap = b_sb[:, 0:1]
        bp1_ap = b_sb[:, 1:2]
        alpha_sb = const.tile([C, 1], f32)
        nc.vector.memset(alpha_sb[:], 0.2)
        zt = const.tile([C, FT], f16)
        nc.vector.memset(zt[:], 0.0)

        xp = ctx.enter_context(tc.tile_pool(name="xp", bufs=3))
        atp = ctx.enter_context(tc.tile_pool(name="atp", bufs=3))
        stp = ctx.enter_context(tc.tile_pool(name="stp", bufs=2))
        nzp = ctx.enter_context(tc.tile_pool(name="nzp", bufs=2))
        Apl = ctx.enter_context(tc.tile_pool(name="Apl", bufs=2))
        qpl = ctx.enter_context(tc.tile_pool(name="qpl", bufs=2))
        tpl = ctx.enter_context(tc.tile_pool(name="tpl", bufs=2))
        ppl = ctx.enter_context(tc.tile_pool(name="ppl", bufs=2))
        prp = ctx.enter_context(tc.tile_pool(name="prp", bufs=2))
        ovp = ctx.enter_context(tc.tile_pool(name="ovp", bufs=3))
        psp = ctx.enter_context(tc.tile_pool(name="psp", bufs=2, space="PSUM"))
        nzps = (
            ctx.enter_context(tc.tile_pool(name="nzps", bufs=2, space="PSUM"))
            if nz_engine == "tensor"
            else None
        )

        ne_eng = {"vector": nc.vector, "gpsimd": nc.gpsimd}.get(ne_engine)
        amul = {"vector": nc.vector, "gpsimd": nc.gpsimd}[amul_engine]

        def issue_loads(sub):
            xt = xp.tile([C, FT], f16, name=f"xt{sub}", tag="xt")
            nc.sync.dma_start(xt[:], x_d[:, sub, :])
            at = atp.tile([C, FT], f16, name=f"at{sub}", tag="at")
            nc.sync.dma_start(at[:], a_d[:, sub, :])
            return xt, at

        def issue_ne(sub, at):
            # (att != 0) for the nz count — issued one subtile early so the
            # tensor engine's nz matmuls never wait on the vector queue.
            st = stp.tile([C, FT], f16, name=f"st{sub}", tag="st")
            if ne_mode == "ttz":
                nc.vector.tensor_tensor(st[:], at[:], zt[:], AL.not_equal)
            else:
                nc.vector.tensor_scalar(st[:], at[:], 0.0, None, AL.not_equal)
            return st

        npre = min(prefetch, NT)
        loaded = {s: issue_loads(s) for s in range(npre)}
        sts = {}
        if nz_engine == "tensor":
            sts[0] = issue_ne(0, loaded[0][1])

        for sub in range(NT):
            xt, at = loaded.pop(sub)
            if sub + npre < NT:
                loaded[sub + npre] = issue_loads(sub + npre)

            # ---- nz = count_nonzero per patch column ----
            nzv = nzp.tile([C, IWT], f32, name=f"nz{sub}", tag="nz")
            if nz_engine == "fold":
                # st = (att != 0) at 4x, then a pairwise fold tree of 2x TT
                # adds over the 81 kernel positions (81 = 2*40 + 1), a tiny
                # strided reduce over the last 5 planes, and the leftover
                # plane folded in at [C, 36] size.
                st = stp.tile([C, FT], f16, name=f"st{sub}", tag="st")
                nc.vector.tensor_scalar(st[:], at[:], 0.0, None, AL.not_equal)
                u1 = stp.tile([C, 40 * IWT], f16, name=f"u1{sub}", tag="u1")
                nc.vector.tensor_tensor(
                    u1[:], st[:, : 40 * IWT], st[:, 40 * IWT : 80 * IWT], AL.add
                )
                u2 = u1[:, : 20 * IWT]
                nc.vector.tensor_tensor(
                    u2, u1[:, : 20 * IWT], u1[:, 20 * IWT : 40 * IWT], AL.add
                )
                u3 = u1[:, : 10 * IWT]
                nc.vector.tensor_tensor(
                    u3, u1[:, : 10 * IWT], u1[:, 10 * IWT : 20 * IWT], AL.add
                )
                u4 = u1[:, : 5 * IWT]
                nc.vector.tensor_tensor(
                    u4, u1[:, : 5 * IWT], u1[:, 5 * IWT : 10 * IWT], AL.add
                )
                nz5 = nzp.tile([C, IWT], f32, name=f"n5{sub}", tag="n5")
                nc.vector.tensor_reduce(
                    nz5[:],
                    u1[:, : 5 * IWT]
                    .rearrange("c (q w) -> c q w", q=5)
                    .transpose([0, 2, 1]),
                    mybir.AxisListType.X,
                    AL.add,
                )
                nc.vector.tensor_tensor(
                    nzv[:], nz5[:], st[:, 80 * IWT : 81 * IWT], AL.add
                )
            elif nz_engine == "tensor":
                # st = (att != 0) on DVE (issued one subtile EARLY, see
                # issue_ne); sum over p1 via 9 accumulated identity matmuls
                # on the tensor engine; sum over p2 with a small 324-element
                # DVE reduce out of PSUM.
                st = sts.pop(sub)
                nzq = nzps.tile([C, PS * IWT], f32)  # 1 bank
                for p1 in range(PS):
                    nc.tensor.matmul(
                        nzq[:],
                        id_sb[:],
                        st[:, p1 * PS * IWT : (p1 + 1) * PS * IWT],
                        start=(p1 == 0),
                        stop=(p1 == PS - 1),
                    )
                nc.vector.tensor_reduce(
                    nzv[:],
                    nzq[:].rearrange("c (q w) -> c q w", q=PS).transpose([0, 2, 1]),
                    mybir.AxisListType.X,
                    AL.add,
                )
            elif nz_engine == "vpad":
                # st padded to pitch 40 (80B rows) so the per-patch reduce
                # walks an aligned stride instead of 72B.
                stw = stp.tile([C, PQ * 40], f16, name=f"sw{sub}", tag="sw")
                st3 = stw[:].rearrange("c (q w) -> c q w", w=40)[:, :, 0:IWT]
                at3 = at[:].rearrange("c (q w) -> c q w", q=PQ)
                if ne_mode == "ttz":
                    zt3 = zt[:].rearrange("c (q w) -> c q w", q=PQ)
                    nc.vector.tensor_tensor(st3, at3, zt3, AL.not_equal)
                else:
                    nc.vector.tensor_scalar(st3, at3, 0.0, None, AL.not_equal)
                nc.vector.tensor_reduce(
                    nzv[:],
                    stw[:].rearrange("c (q w) -> c q w", w=40)[:, :, 0:IWT]
                    .transpose([0, 2, 1]),
                    mybir.AxisListType.X,
                    AL.add,
                )
            elif ne_engine == "scalar":
                # |sign(att)| summed with absolute-value reduce
                st = stp.tile([C, FT], f16, name=f"st{sub}", tag="st")
                nc.scalar.activation(st[:], at[:], AF.Sign)
                nc.vector.tensor_reduce(
                    nzv[:],
                    st[:].rearrange("c (q w) -> c q w", q=PQ).transpose([0, 2, 1]),
                    mybir.AxisListType.X,
                    AL.add,
                    apply_absolute_value=True,
                )
            else:
                st = stp.tile([C, FT], f16, name=f"st{sub}", tag="st")
                ne_eng.tensor_scalar(st[:], at[:], 0.0, None, AL.not_equal)
                nc.vector.tensor_reduce(
                    nzv[:],
                    st[:].rearrange("c (q w) -> c q w", q=PQ).transpose([0, 2, 1]),
                    mybir.AxisListType.X,
                    AL.add,
                )
            # r = 1/nz in fp16 (the +1e-5 of the reference shifts r by
            # ~1.2e-7 relative — far below fp16 rounding, so it is dropped)
            rcp = nzp.tile([C, IWT], f32, name=f"rc{sub}", tag="rc")
            nc.vector.reciprocal_approx_fast(rcp[:], nzv[:])
            rh = nzp.tile([C, IWT], f16, name=f"rh{sub}", tag="rh")
            if rh_engine == "scalar":
                # tiny f32->f16 cast on the otherwise-idle scalar engine:
                # back-to-back dependent DVE ops stall ~1.4us regardless of
                # size, so keeping rcp->rh off one queue breaks the chain
                nc.scalar.copy(rh[:], rcp[:])
            else:
                nc.vector.tensor_scalar(rh[:], rcp[:], 0.0, None, AL.add)

            # ---- A = att * r  (r broadcast over the 81 kernel positions) ----
            At = Apl.tile([C, FT], f16, name=f"A{sub}", tag="A")
            a3 = at[:].rearrange("c (q w) -> c q w", q=PQ)
            r3 = rh[:].unsqueeze(1).broadcast_to((C, PQ, IWT))
            amul.tensor_tensor(
                At[:].rearrange("c (q w) -> c q w", q=PQ), a3, r3, AL.mult
            )

            # ---- GEMM: psum = W @ x ----
            pst = []
            for g in range(NGRP):
                ps_t = psp.tile([C, MMG * BANK], f32)  # 3 banks
                pst.append(ps_t)
                for m in range(MMG):
                    ch = g * MMG + m
                    nc.tensor.matmul(
                        ps_t[:, m * BANK : m * BANK + MMN],
                        wt_sb[:],
                        xt[:, ch * MMN : (ch + 1) * MMN],
                        start=True,
                        stop=True,
                    )

            # next subtile's (att != 0) goes on the vector queue BEFORE this
            # subtile's fused tail so the tensor engine's nz matmuls for
            # sub+1 are never blocked behind OP_PRE in vector program order
            if nz_engine == "tensor" and sub + 1 < NT:
                sts[sub + 1] = issue_ne(sub + 1, loaded[sub + 1][1])

            ov = ovp.tile([C, FT], f16, name=f"ov{sub}", tag="ov")
            if fuse == "dve":
                # ---- ov = lrelu((ps+b+1+A)*(ps+b)) fused, one op/group ----
                for g in range(NGRP):
                    ps_ap = (
                        pst[g][:]
                        .rearrange("c (m n) -> c m n", n=BANK)[:, :, 0:MMN]
                    )
                    sl = slice(g * MMG * MMN, (g + 1) * MMG * MMN)
                    nc.vector._custom_dve(
                        op_pre,
                        out=ov[:, sl],
                        in0=ps_ap,
                        in1=At[:, sl],
                        s0=bp1_ap,
                        s1=1.0,
                        imm2=0.2,
                    )
            else:
                # ---- q = psum + (b+1), PSUM -> fp16 SBUF (scalar engine) ---
                qt = qpl.tile([C, FT], f16, name=f"q{sub}", tag="q")
                for g in range(NGRP):
                    ps_ap = (
                        pst[g][:]
                        .rearrange("c (m n) -> c m n", n=BANK)[:, :, 0:MMN]
                    )
                    q_ap = qt[:, g * MMG * MMN : (g + 1) * MMG * MMN].rearrange(
                        "c (m n) -> c m n", n=MMN
                    )
                    nc.scalar.activation(q_ap, ps_ap, AF.Identity, bias=bp1_ap)

                # ---- t = A + q ; p = q - 1 ; pre = t * p ----
                tt = tpl.tile([C, FT], f16, name=f"t{sub}", tag="t")
                nc.vector.tensor_tensor(tt[:], At[:], qt[:], AL.add)
                pt = ppl.tile([C, FT], f16, name=f"p{sub}", tag="p")
                if p_engine == "vector":
                    nc.vector.tensor_scalar(pt[:], qt[:], 1.0, None, AL.subtract)
                else:
                    nc.scalar.activation(pt[:], qt[:], AF.Identity, bias=-1.0)
                pre = prp.tile([C, FT], f16, name=f"pr{sub}", tag="pr")
                nc.vector.tensor_tensor(pre[:], tt[:], pt[:], AL.mult)

                # ---- out = lrelu(pre) ----
                if prelu_engine == "scalar":
                    nc.scalar.activation(
                        ov[:], pre[:], AF.Prelu, alpha=alpha_sb[:, 0:1]
                    )
                else:
                    nc.vector.scalar_tensor_tensor(
                        ov[:], pre[:], 0.2, pre[:], AL.mult, AL.max
                    )

            nc.sync.dma_start(o_d[:, sub, :], ov[:])

    nc.compile()
    return nc


def _get_nc(**kw):
    key = tuple(sorted(kw.items()))
    if key not in _NC_CACHE:
        _NC_CACHE[key] = _build_nc(**kw)
    return _NC_CACHE[key]


def kernel(x, attentions, W, b, **build_kw):
    global LAST_RESULT
    from concourse.bass_utils import run_bass_kernel_spmd

    x = np.asarray(x, dtype=np.float32)
    attentions = np.asarray(attentions, dtype=np.float32)
    W = np.asarray(W, dtype=np.float32)
    b = np.asarray(b, dtype=np.float32)

    nc = _get_nc(**build_kw)

    # x: [1, C, D, HW] -> (c, iD, p1, s, h, iw, p2) -> per-core (c, iD, h, p1, p2, iw)
    xs = x.reshape(C, NDP, PS, NCORES, NSUB, IWT, PS)
    # att: [1, C, L, 81] with L=(iD, s, h, iw), 81=(p1, p2)
    as_ = attentions.reshape(C, NDP, NCORES, NSUB, IWT, PS, PS)
    wt = np.ascontiguousarray(W.T.astype(np.float16))
    ident = np.eye(C, dtype=np.float16)
    bcol = np.ascontiguousarray(np.stack([b, b + 1.0], axis=1))

    in_maps = []
    for s in range(NCORES):
        xc = xs[:, :, :, s].transpose(0, 1, 3, 2, 5, 4)  # c,iD,h,p1,p2,iw
        ac = as_[:, :, s].transpose(0, 1, 2, 4, 5, 3)    # c,iD,h,p1,p2,iw
        in_maps.append(
            {
                "x": np.ascontiguousarray(xc, dtype=np.float16).reshape(C, NT, FT),
                "att": np.ascontiguousarray(ac, dtype=np.float16).reshape(C, NT, FT),
                "wt": wt,
                "ident": ident,
                "bias": bcol,
            }
        )

    res = run_bass_kernel_spmd(
        nc,
        in_maps,
        core_ids=list(range(NCORES)),
        trace=bool(os.environ.get("BASS_TRACE")),
    )
    LAST_RESULT = res

    # out: per-core [C, NT, FT] = (c, iD, h, p1, p2, iw) -> [1, C, D, HW]
    full = np.empty((C, NDP, PS, NCORES, NSUB, IWT, PS), dtype=np.float32)
    for s in range(NCORES):
        oc = res.results[s]["out"].reshape(C, NDP, NSUB, PS, PS, IWT)
        full[:, :, :, s] = oc.transpose(0, 1, 3, 2, 5, 4).astype(np.float32)
    return full.reshape(1, C, D, HWFULL)


# revision 29
# speedup vs baseline: 1.3612x; 1.0238x over previous
"""Trainium2 Bass kernel for nn_Cross_head (sparse_attention patch-correction).

Math (non-overlapping unfold/fold are inverse permutations):
    y   = W @ x + b                   (1x1x1 conv over channels)
    out = leaky_relu(y * (y + 1 + A), 0.2),  A = att / (count_nonzero(att) + 1e-5)

Factorization used on device (q read once from PSUM by the scalar engine):
    q   = psum + (b+1)        # scalar engine, PSUM -> SBUF fp16
    A   = att * r             # r = 1/nz broadcast per patch column
    t   = A + q               # = y + 1 + A
    p   = q - 1               # = y
    pre = t * p
    out = prelu(pre, 0.2)

Sharding: spatial across the 576 patch columns (72 per core), no cross-core
communication.  All I/O is fp16 (host casts/packs), every DMA is contiguous
per channel (5832B descriptors), pure HWDGE on the sync queue.

Per-subtile free layout is (pq=81, p2-major inside, iw=36) so that every
element-wise operand is innermost-packed fp16 in SBUF: TT ops run in DVE 2x
mode, TS ops in 4x mode.  The 1/nz broadcast operand is packed on its
innermost (iw) dim with stride-0 only on the outer pq dim, which keeps 2x.
The nz count-reduce reads st=(att!=0) with a strided view (no fast mode for
reduce anyway).
"""

import os
import sys

import numpy as np

sys.path.insert(0, "/opt/trn_rl_repo")

# ---- geometry (hardcoded for this problem) ----
C = 128          # channels (in == out)
D = 36           # depth
HWFULL = 5184    # H*W = 72*72
PS = 9           # patch size
PQ = PS * PS     # 81 kernel positions
NDP = 4          # D // PS
NWP = 576        # HWFULL // PS  (patch columns)
NCORES = 8
IWG = NWP // NCORES   # 72 patch columns per core
NSUB = 2              # split each iD block into halves along iW
IWT = IWG // NSUB     # 36 patch columns per subtile
FT = IWT * PQ         # 2916 elements per subtile per partition
NT = NDP * NSUB       # 8 subtiles
MMN = 486             # matmul free dim (2916 / 6)
NMM = 6               # matmuls per subtile
NGRP = 2              # psum groups per subtile
MMG = NMM // NGRP     # 3 matmuls per psum group
BANK = 512            # fp32 elements per PSUM bank

_NC_CACHE = {}
LAST_RESULT = None

_OP_PRE = None


def _get_op_pre():
    """Register (once) a custom DVE op that fuses the whole element-wise
    tail into a single pass reading PSUM:
        z0  = in0 + s0          # psum + (b+1)  -> y + 1 + A-part. . .
        z   = (z0 + in1) * (z0 - s1)   # (y+1+A) * y   [s1 = 1]
        out = max(z, z*imm2)           # leaky_relu    [imm2 = 0.2]
    This replaces the q/t/p/pre intermediate tiles (and the scalar-engine
    Prelu) with one DVE instruction per PSUM group, saving ~45KB/partition
    of SBUF traffic per subtile.  Uses the documented custom-DVE extension
    point (dve_ops.OPS)."""
    global _OP_PRE
    if _OP_PRE is not None:
        return _OP_PRE
    import numpy as np

    from concourse import dve_ops
    from concourse.dve_spec import C0, C1, C2, Spec, Src0, Src1, lower, maxx
    from concourse.dve_uop import DveOpSpec

    NAME = "CROSS_HEAD_PRE_LRELU"
    for op in dve_ops.OPS:
        if op.name == NAME:
            _OP_PRE = op
            return op

    _z0 = Src0 + C0
    _z = (_z0 + Src1) * (_z0 - C1)
    body = maxx(_z, _z * C2)

    def _ref(in0, in1, s0, s1, imm2):
        p = in0.shape[0]
        a = np.asarray(in0, np.float32).reshape(p, -1)
        b = np.asarray(in1, np.float32).reshape(p, -1)
        z0 = a + s0
        z = (z0 + b) * (z0 - s1)
        return np.maximum(z, z * imm2).astype(np.float32)

    spec = Spec(body=body, reference=_ref)
    row = max(dve_ops._SUB_OPCODE_FOR_NAME.values()) + 1
    assert row < 0x20
    dve_ops._SUB_OPCODE_FOR_NAME[NAME] = row
    shas = {}
    for ver in ("v3", "v4"):
        uops = lower(spec, ver=ver)
        shas[ver] = DveOpSpec(
            name=NAME, opcode=row, uops=uops, rd1_en=True
        ).sha(ver)
    op = dve_ops.DveOp(NAME, spec, subdim=False, uops_sha=shas)
    dve_ops.OPS.append(op)
    dve_ops.CUSTOM_DVE_SPECS[NAME] = spec
    _OP_PRE = op
    return op


def _build_nc(ne_engine="vector", amul_engine="gpsimd", prelu_engine="scalar",
              p_engine="vector", nz_engine="tensor", ne_mode="ts", fuse="dve",
              rh_engine="scalar", prefetch=4, amul_tail=None):
    from contextlib import ExitStack

    import concourse.bacc as bacc
    import concourse.tile as tile
    from concourse import mybir

    f32 = mybir.dt.float32
    f16 = mybir.dt.float16
    AL = mybir.AluOpType
    AF = mybir.ActivationFunctionType

    op_pre = _get_op_pre() if fuse == "dve" else None

    nc = bacc.Bacc(
        "TRN2",
        target_bir_lowering=False,
        debug=False,
        enable_asserts=False,
        num_devices=NCORES,
    )
    x_d = nc.dram_tensor("x", [C, NT, FT], f16, kind="ExternalInput").ap()
    a_d = nc.dram_tensor("att", [C, NT, FT], f16, kind="ExternalInput").ap()
    wt_d = nc.dram_tensor("wt", [C, C], f16, kind="ExternalInput").ap()
    id_d = nc.dram_tensor("ident", [C, C], f16, kind="ExternalInput").ap()
    b_d = nc.dram_tensor("bias", [C, 2], f32, kind="ExternalInput").ap()
    o_d = nc.dram_tensor("out", [C, NT, FT], f16, kind="ExternalOutput").ap()

    with tile.TileContext(nc) as tc, ExitStack() as ctx:
        const = ctx.enter_context(tc.tile_pool(name="const", bufs=1))
        wt_sb = const.tile([C, C], f16)
        nc.sync.dma_start(wt_sb[:], wt_d[:])
        id_sb = const.tile([C, C], f16)
        nc.sync.dma_start(id_sb[:], id_d[:])
        b_sb = const.tile([C, 2], f32)
        nc.sync.dma_start(b_sb[:], b_d[:])
        b_ap = b_sb[:, 0:1]
        bp1_ap = b_sb[:, 1:2]
        alpha_sb = const.tile([C, 1], f32)
        nc.vector.memset(alpha_sb[:], 0.2)
        zt = const.tile([C, FT], f16)
        nc.vector.memset(zt[:], 0.0)

        xp = ctx.enter_context(tc.tile_pool(name="xp", bufs=3))
        atp = ctx.enter_context(tc.tile_pool(name="atp", bufs=3))
        stp = ctx.enter_context(tc.tile_pool(name="stp", bufs=2))
        nzp = ctx.enter_context(tc.tile_pool(name="nzp", bufs=2))
        Apl = ctx.enter_context(tc.tile_pool(name="Apl", bufs=2))
        qpl = ctx.enter_context(tc.tile_pool(name="qpl", bufs=2))
        tpl = ctx.enter_context(tc.tile_pool(name="tpl", bufs=2))
        ppl = ctx.enter_context(tc.tile_pool(name="ppl", bufs=2))
        prp = ctx.enter_context(tc.tile_pool(name="prp", bufs=2))
        ovp = ctx.enter_context(tc.tile_pool(name="ovp", bufs=3))
        psp = ctx.enter_context(tc.tile_pool(name="psp", bufs=2, space="PSUM"))
        nzps = (
            ctx.enter_context(tc.tile_pool(name="nzps", bufs=2, space="PSUM"))
            if nz_engine == "tensor"
            else None
        )

        ne_eng = {"vector": nc.vector, "gpsimd": nc.gpsimd}.get(ne_engine)
        amul = {"vector": nc.vector, "gpsimd": nc.gpsimd}[amul_engine]

        def issue_loads(sub):
            xt = xp.tile([C, FT], f16, name=f"xt{sub}", tag="xt")
            nc.sync.dma_start(xt[:], x_d[:, sub, :])
            at = atp.tile([C, FT], f16, name=f"at{sub}", tag="at")
            nc.sync.dma_start(at[:], a_d[:, sub, :])
            return xt, at

        def issue_ne(sub, at):
            # (att != 0) for the nz count — issued one subtile early so the
            # tensor engine's nz matmuls never wait on the vector queue.
            st = stp.tile([C, FT], f16, name=f"st{sub}", tag="st")
            if ne_mode == "ttz":
                nc.vector.tensor_tensor(st[:], at[:], zt[:], AL.not_equal)
            else:
                nc.vector.tensor_scalar(st[:], at[:], 0.0, None, AL.not_equal)
            return st

        npre = min(prefetch, NT)
        loaded = {s: issue_loads(s) for s in range(npre)}
        sts = {}
        if nz_engine == "tensor":
            sts[0] = issue_ne(0, loaded[0][1])

        for sub in range(NT):
            xt, at = loaded.pop(sub)
            if sub + npre < NT:
                loaded[sub + npre] = issue_loads(sub + npre)

            # ---- nz = count_nonzero per patch column ----
            nzv = nzp.tile([C, IWT], f32, name=f"nz{sub}", tag="nz")
            if nz_engine == "fold":
                # st = (att != 0) at 4x, then a pairwise fold tree of 2x TT
                # adds over the 81 kernel positions (81 = 2*40 + 1), a tiny
                # strided reduce over the last 5 planes, and the leftover
                # plane folded in at [C, 36] size.
                st = stp.tile([C, FT], f16, name=f"st{sub}", tag="st")
                nc.vector.tensor_scalar(st[:], at[:], 0.0, None, AL.not_equal)
                u1 = stp.tile([C, 40 * IWT], f16, name=f"u1{sub}", tag="u1")
                nc.vector.tensor_tensor(
                    u1[:], st[:, : 40 * IWT], st[:, 40 * IWT : 80 * IWT], AL.add
                )
                u2 = u1[:, : 20 * IWT]
                nc.vector.tensor_tensor(
                    u2, u1[:, : 20 * IWT], u1[:, 20 * IWT : 40 * IWT], AL.add
                )
                u3 = u1[:, : 10 * IWT]
                nc.vector.tensor_tensor(
                    u3, u1[:, : 10 * IWT], u1[:, 10 * IWT : 20 * IWT], AL.add
                )
                u4 = u1[:, : 5 * IWT]
                nc.vector.tensor_tensor(
                    u4, u1[:, : 5 * IWT], u1[:, 5 * IWT : 10 * IWT], AL.add
                )
                nz5 = nzp.tile([C, IWT], f32, name=f"n5{sub}", tag="n5")
                nc.vector.tensor_reduce(
                    nz5[:],
                    u1[:, : 5 * IWT]
                    .rearrange("c (q w) -> c q w", q=5)
                    .transpose([0, 2, 1]),
                    mybir.AxisListType.X,
                    AL.add,
                )
                nc.vector.tensor_tensor(
                    nzv[:], nz5[:], st[:, 80 * IWT : 81 * IWT], AL.add
                )
            elif nz_engine == "tensor":
                # st = (att != 0) on DVE (issued one subtile EARLY, see
                # issue_ne); sum over p1 via 9 accumulated identity matmuls
                # on the tensor engine; sum over p2 with a small 324-element
                # DVE reduce out of PSUM.
                st = sts.pop(sub)
                nzq = nzps.tile([C, PS * IWT], f32)  # 1 bank
                for p1 in range(PS):
                    nc.tensor.matmul(
                        nzq[:],
                        id_sb[:],
                        st[:, p1 * PS * IWT : (p1 + 1) * PS * IWT],
                        start=(p1 == 0),
                        stop=(p1 == PS - 1),
                    )
                nc.vector.tensor_reduce(
                    nzv[:],
                    nzq[:].rearrange("c (q w) -> c q w", q=PS).transpose([0, 2, 1]),
                    mybir.AxisListType.X,
                    AL.add,
                )
            elif nz_engine == "vpad":
                # st padded to pitch 40 (80B rows) so the per-patch reduce
                # walks an aligned stride instead of 72B.
                stw = stp.tile([C, PQ * 40], f16, name=f"sw{sub}", tag="sw")
                st3 = stw[:].rearrange("c (q w) -> c q w", w=40)[:, :, 0:IWT]
                at3 = at[:].rearrange("c (q w) -> c q w", q=PQ)
                if ne_mode == "ttz":
                    zt3 = zt[:].rearrange("c (q w) -> c q w", q=PQ)
                    nc.vector.tensor_tensor(st3, at3, zt3, AL.not_equal)
                else:
                    nc.vector.tensor_scalar(st3, at3, 0.0, None, AL.not_equal)
                nc.vector.tensor_reduce(
                    nzv[:],
                    stw[:].rearrange("c (q w) -> c q w", w=40)[:, :, 0:IWT]
                    .transpose([0, 2, 1]),
                    mybir.AxisListType.X,
                    AL.add,
                )
            elif ne_engine == "scalar":
                # |sign(att)| summed with absolute-value reduce
                st = stp.tile([C, FT], f16, name=f"st{sub}", tag="st")
                nc.scalar.activation(st[:], at[:], AF.Sign)
                nc.vector.tensor_reduce(
                    nzv[:],
                    st[:].rearrange("c (q w) -> c q w", q=PQ).transpose([0, 2, 1]),
                    mybir.AxisListType.X,
                    AL.add,
                    apply_absolute_value=True,
                )
            else:
                st = stp.tile([C, FT], f16, name=f"st{sub}", tag="st")
                ne_eng.tensor_scalar(st[:], at[:], 0.0, None, AL.not_equal)
                nc.vector.tensor_reduce(
                    nzv[:],
                    st[:].rearrange("c (q w) -> c q w", q=PQ).transpose([0, 2, 1]),
                    mybir.AxisListType.X,
                    AL.add,
                )
            # r = 1/nz in fp16 (the +1e-5 of the reference shifts r by
            # ~1.2e-7 relative — far below fp16 rounding, so it is dropped)
            rcp = nzp.tile([C, IWT], f32, name=f"rc{sub}", tag="rc")
            nc.vector.reciprocal_approx_fast(rcp[:], nzv[:])
            rh = nzp.tile([C, IWT], f16, name=f"rh{sub}", tag="rh")
            if rh_engine == "scalar":
                # tiny f32->f16 cast on the otherwise-idle scalar engine:
                # back-to-back dependent DVE ops stall ~1.4us regardless of
                # size, so keeping rcp->rh off one queue breaks the chain
                nc.scalar.copy(rh[:], rcp[:])
            else:
                nc.vector.tensor_scalar(rh[:], rcp[:], 0.0, None, AL.add)

            # ---- A = att * r  (r broadcast over the 81 kernel positions) ----
            At = Apl.tile([C, FT], f16, name=f"A{sub}", tag="A")
            a3 = at[:].rearrange("c (q w) -> c q w", q=PQ)
            r3 = rh[:].unsqueeze(1).broadcast_to((C, PQ, IWT))
            # the final subtile's A has nothing left to overlap: gpsimd's
            # slow TT (5.7us) plus two cross-engine handoffs sit on the
            # drain path, so route it to vector (1.7us, same queue as the
            # fused tail) when amul_tail says so
            amul_s = amul
            if amul_tail is not None and sub == NT - 1:
                amul_s = {"vector": nc.vector, "gpsimd": nc.gpsimd}[amul_tail]
            amul_s.tensor_tensor(
                At[:].rearrange("c (q w) -> c q w", q=PQ), a3, r3, AL.mult
            )

            # ---- GEMM: psum = W @ x ----
            pst = []
            for g in range(NGRP):
                ps_t = psp.tile([C, MMG * BANK], f32)  # 3 banks
                pst.append(ps_t)
                for m in range(MMG):
                    ch = g * MMG + m
                    nc.tensor.matmul(
                        ps_t[:, m * BANK : m * BANK + MMN],
                        wt_sb[:],
                        xt[:, ch * MMN : (ch + 1) * MMN],
                        start=True,
                        stop=True,
                    )

            # next subtile's (att != 0) goes on the vector queue BEFORE this
            # subtile's fused tail so the tensor engine's nz matmuls for
            # sub+1 are never blocked behind OP_PRE in vector program order
            if nz_engine == "tensor" and sub + 1 < NT:
                sts[sub + 1] = issue_ne(sub + 1, loaded[sub + 1][1])

            ov = ovp.tile([C, FT], f16, name=f"ov{sub}", tag="ov")
            if fuse == "dve":
                # ---- ov = lrelu((ps+b+1+A)*(ps+b)) fused, one op/group ----
                for g in range(NGRP):
                    ps_ap = (
                        pst[g][:]
                        .rearrange("c (m n) -> c m n", n=BANK)[:, :, 0:MMN]
                    )
                    sl = slice(g * MMG * MMN, (g + 1) * MMG * MMN)
                    nc.vector._custom_dve(
                        op_pre,
                        out=ov[:, sl],
                        in0=ps_ap,
                        in1=At[:, sl],
                        s0=bp1_ap,
                        s1=1.0,
                        imm2=0.2,
                    )
            else:
                # ---- q = psum + (b+1), PSUM -> fp16 SBUF (scalar engine) ---
                qt = qpl.tile([C, FT], f16, name=f"q{sub}", tag="q")
                for g in range(NGRP):
                    ps_ap = (
                        pst[g][:]
                        .rearrange("c (m n) -> c m n", n=BANK)[:, :, 0:MMN]
                    )
                    q_ap = qt[:, g * MMG * MMN : (g + 1) * MMG * MMN].rearrange(
                        "c (m n) -> c m n", n=MMN
                    )
                    nc.scalar.activation(q_ap, ps_ap, AF.Identity, bias=bp1_ap)

                # ---- t = A + q ; p = q - 1 ; pre = t * p ----
                tt = tpl.tile([C, FT], f16, name=f"t{sub}", tag="t")
                nc.vector.tensor_tensor(tt[:], At[:], qt[:], AL.add)
                pt = ppl.tile([C, FT], f16, name=f"p{sub}", tag="p")
                if p_engine == "vector":
                    nc.vector.tensor_scalar(pt[:], qt[:], 1.0, None, AL.subtract)
                else:
                    nc.scalar.activation(pt[:], qt[:], AF.Identity, bias=-1.0)
                pre = prp.tile([C, FT], f16, name=f"pr{sub}", tag="pr")
                nc.vector.tensor_tensor(pre[:], tt[:], pt[:], AL.mult)

                # ---- out = lrelu(pre) ----
                if prelu_engine == "scalar":
                    nc.scalar.activation(
                        ov[:], pre[:], AF.Prelu, alpha=alpha_sb[:, 0:1]
                    )
                else:
                    nc.vector.scalar_tensor_tensor(
                        ov[:], pre[:], 0.2, pre[:], AL.mult, AL.max
                    )

            nc.sync.dma_start(o_d[:, sub, :], ov[:])

    nc.compile()
    return nc


def _get_nc(**kw):
    key = tuple(sorted(kw.items()))
    if key not in _NC_CACHE:
        _NC_CACHE[key] = _build_nc(**kw)
    return _NC_CACHE[key]


def kernel(x, attentions, W, b, **build_kw):
    global LAST_RESULT
    from concourse.bass_utils import run_bass_kernel_spmd

    x = np.asarray(x, dtype=np.float32)
    attentions = np.asarray(attentions, dtype=np.float32)
    W = np.asarray(W, dtype=np.float32)
    b = np.asarray(b, dtype=np.float32)

    nc = _get_nc(**build_kw)

    # x: [1, C, D, HW] -> (c, iD, p1, s, h, iw, p2) -> per-core (c, iD, h, p1, p2, iw)
    xs = x.reshape(C, NDP, PS, NCORES, NSUB, IWT, PS)
    # att: [1, C, L, 81] with L=(iD, s, h, iw), 81=(p1, p2)
    as_ = attentions.reshape(C, NDP, NCORES, NSUB, IWT, PS, PS)
    wt = np.ascontiguousarray(W.T.astype(np.float16))
    ident = np.eye(C, dtype=np.float16)
    bcol = np.ascontiguousarray(np.stack([b, b + 1.0], axis=1))

    in_maps = []
    for s in range(NCORES):
        xc = xs[:, :, :, s].transpose(0, 1, 3, 2, 5, 4)  # c,iD,h,p1,p2,iw
        ac = as_[:, :, s].transpose(0, 1, 2, 4, 5, 3)    # c,iD,h,p1,p2,iw
        in_maps.append(
            {
                "x": np.ascontiguousarray(xc, dtype=np.float16).reshape(C, NT, FT),
                "att": np.ascontiguousarray(ac, dtype=np.float16).reshape(C, NT, FT),
                "wt": wt,
                "ident": ident,
                "bias": bcol,
            }
        )

    res = run_bass_kernel_spmd(
        nc,
        in_maps,
        core_ids=list(range(NCORES)),
        trace=bool(os.environ.get("BASS_TRACE")),
    )
    LAST_RESULT = res

    # out: per-core [C, NT, FT] = (c, iD, h, p1, p2, iw) -> [1, C, D, HW]
    full = np.empty((C, NDP, PS, NCORES, NSUB, IWT, PS), dtype=np.float32)
    for s in range(NCORES):
        oc = res.results[s]["out"].reshape(C, NDP, NSUB, PS, PS, IWT)
        full[:, :, :, s] = oc.transpose(0, 1, 3, 2, 5, 4).astype(np.float32)
    return full.reshape(1, C, D, HWFULL)
